# revision 1
# baseline (speedup 1.0000x reference)
"""Trainium2 Bass kernel for a pre-LN transformer block (causal MHA + GELU FFN).

Problem: x[64, 512, 384], 7 heads x 54, FFN 2304. Sharded data-parallel over
batch across 8 NeuronCores (8 batches/core); no collectives needed.

Per-core dataflow (token-major <-> feature-major via PE transposes):
  LN1 (bn_stats)  -> xn tiles -> PE-transpose -> xnT [384, tok]
  QKV: Q^T/K^T feature-major per head-pair (partitions 0-53 / 64-117),
       V token-major with a ones-column per head (unnormalized-softmax trick)
  scores^T[t,s] per (b,h) -> exp (ACT, psum->sbuf) -> diag causal mask (DVE)
  o^T[55, s] = V_hat^T @ attn^T  (row 0 = softmax denominators)
  normalize = tensor_mul by gpsimd-broadcast reciprocal  (the psum->sbuf copy)
  att = O^T @ Wo (+x residual) -> x2 (DRAM bounce); LN2 -> xn2T; FFN1 ->
  gelu(+b1) -> FFN2 (two 9-chunk groups accumulating in one PSUM bank) ->
  out = x2 + ffn
All matmuls run as float32r (full PE rate at free-dim >= 256), fp32 accumulate.
"""

import numpy as np
from contextlib import ExitStack

import concourse.bass as bass
import concourse.bacc as bacc
import concourse.mybir as mybir
import concourse.tile as tile
from concourse import masks
from concourse.bass_utils import run_bass_kernel_spmd

# ---- problem constants (hardcoded per harness contract) ----
B, S, D = 64, 512, 384
H, HS = 7, 54
FFN = 6 * D  # 2304
EPS = 1e-5
N_CORES = 8
B_LOC = B // N_CORES          # 8 batches per core
F32 = mybir.dt.float32
F32R = mybir.dt.float32r
AF = mybir.ActivationFunctionType
ALU = mybir.AluOpType

N_D = D // 128                # 3 d-chunks
N_F = FFN // 128              # 18 ffn-chunks
PAIRS = (H + 1) // 2          # 4 head-pair groups (last has 1 head)

LAST_RESULTS = None


def _rr(ap):
    """bitcast an SBUF ap to float32r for full-rate PE streaming"""
    return ap.bitcast(F32R)


def to_fp32r(a):
    """Round fp32 to the fp32r format walrus expects for DMA'd matmul
    operands: 11-bit mantissa, RNE, low 12 bits zeroed."""
    u = np.ascontiguousarray(a, np.float32).view(np.uint32)
    r = (u + np.uint32(0x7FF) + ((u >> np.uint32(12)) & np.uint32(1))) & np.uint32(0xFFFFF000)
    return r.view(np.float32)


def build_program(n_b=B_LOC, has_bias_o=False, has_bias_2=False, mm_dt="f32r",
                  n_reps=1):
    nc = bacc.Bacc()
    NTOK = n_b * S
    NT = NTOK // 128          # token tiles per core

    cast = _rr if mm_dt == "f32r" else (lambda ap: ap)
    MDT = F32R if mm_dt == "f32r" else F32   # dtype of matmul-feeding tiles
    x_d = nc.declare_dram_parameter("x", [NTOK, D], F32, isOutput=False)
    wq_d = nc.declare_dram_parameter("wq_pad", [D, 512], MDT, isOutput=False)
    wk_d = nc.declare_dram_parameter("wk_pad", [D, 512], MDT, isOutput=False)
    wv_d = nc.declare_dram_parameter("wv_pad", [D, 512], MDT, isOutput=False)
    wo_d = nc.declare_dram_parameter("wo_pad", [H, 55, D], MDT, isOutput=False)
    w1_d = nc.declare_dram_parameter("w1", [D, FFN], MDT, isOutput=False)
    w2_d = nc.declare_dram_parameter("w2", [FFN, D], MDT, isOutput=False)
    b1_d = nc.declare_dram_parameter("b1c", [128, N_F], F32, isOutput=False)
    bo_d = nc.declare_dram_parameter("bo", [1, D], MDT, isOutput=False)
    b2_d = nc.declare_dram_parameter("b2", [1, D], MDT, isOutput=False)
    out_d = nc.declare_dram_parameter("out", [NTOK, D], F32, isOutput=True)

    with tile.TileContext(nc) as tc, ExitStack() as ctx:
        # ---------------- persistent pools ----------------
        wpool = ctx.enter_context(tc.tile_pool(name="weights", bufs=1))
        wq_sb = [wpool.tile([128, 512], MDT, tag=f"wq{d}", name=f"wq{d}") for d in range(N_D)]
        wk_sb = [wpool.tile([128, 512], MDT, tag=f"wk{d}", name=f"wk{d}") for d in range(N_D)]
        wv_sb = [wpool.tile([128, 512], MDT, tag=f"wv{d}", name=f"wv{d}") for d in range(N_D)]
        wo_sb = [wpool.tile([55, D], MDT, tag=f"wo{h}", name=f"wo{h}") for h in range(H)]
        w1_sb = [wpool.tile([128, FFN], MDT, tag=f"w1{d}", name=f"w1{d}") for d in range(N_D)]
        w2_sb = [wpool.tile([128, D], MDT, tag=f"w2{f}", name=f"w2{f}") for f in range(N_F)]
        b1_sb = wpool.tile([128, N_F], F32, tag="b1")
        bo_sb = wpool.tile([1, D], MDT, tag="bo")
        b2_sb = wpool.tile([1, D], MDT, tag="b2")
        eps_sb = wpool.tile([128, 1], F32, tag="eps")
        ones_sb = wpool.tile([1, 128], MDT, tag="ones")
        trimask = wpool.tile([128, 128], F32, tag="trimask")
        identity = wpool.tile([128, 128], F32, tag="ident")

        for d in range(N_D):
            nc.sync.dma_start(wq_sb[d][:], wq_d[128 * d:128 * (d + 1), :])
            nc.sync.dma_start(wk_sb[d][:], wk_d[128 * d:128 * (d + 1), :])
            nc.sync.dma_start(wv_sb[d][:], wv_d[128 * d:128 * (d + 1), :])
            nc.sync.dma_start(w1_sb[d][:], w1_d[128 * d:128 * (d + 1), :])
        for h in range(H):
            nc.sync.dma_start(wo_sb[h][:], wo_d[h])
        for f in range(N_F):
            nc.sync.dma_start(w2_sb[f][:], w2_d[128 * f:128 * (f + 1), :])
        nc.sync.dma_start(b1_sb[:], b1_d[:])
        nc.sync.dma_start(bo_sb[:], bo_d[:])
        nc.sync.dma_start(b2_sb[:], b2_d[:])
        nc.any.memset(eps_sb[:], EPS)
        nc.any.memset(ones_sb[:].bitcast(F32), 1.0)
        masks.make_identity(nc, identity[:])
        # trimask[t, s] = 1.0 if s >= t else 0.0  (upper triangular incl diag)
        masks.make_upper_triangular(nc, trimask[:], val=1.0, diag=True)

        # x2 bounce buffer in DRAM (SBUF is tight)
        dpool = ctx.enter_context(tc.tile_pool(name="dram", bufs=1, space="DRAM"))
        x2_ds = [dpool.tile([NTOK, D], F32, tag=f"x2d{r}", name=f"x2d{r}")
                 for r in range(n_reps)]
        chain = [dpool.tile([NTOK, D], F32, tag=f"chain{i}", name=f"chain{i}")
                 for i in range(max(n_reps - 1, 0))]

        # ---------------- streaming pools ----------------
        xpool = ctx.enter_context(tc.tile_pool(name="xin", bufs=4))
        stpool = ctx.enter_context(tc.tile_pool(name="stats", bufs=4))
        xnpool = ctx.enter_context(tc.tile_pool(name="xn", bufs=4))
        xTpool = ctx.enter_context(tc.tile_pool(name="xT", bufs=2))
        qkpool = ctx.enter_context(tc.tile_pool(name="qk", bufs=1))
        vpool = ctx.enter_context(tc.tile_pool(name="v", bufs=4))
        epool = ctx.enter_context(tc.tile_pool(name="expT", bufs=2))
        rpool = ctx.enter_context(tc.tile_pool(name="recip", bufs=2))
        otpool = ctx.enter_context(tc.tile_pool(name="oT", bufs=1))
        hpool = ctx.enter_context(tc.tile_pool(name="hgelu", bufs=10))
        opool = ctx.enter_context(tc.tile_pool(name="outt", bufs=4))

        ps_proj = ctx.enter_context(tc.tile_pool(name="ps_proj", bufs=3, space="PSUM"))
        ps_sc = ctx.enter_context(tc.tile_pool(name="ps_sc", bufs=1, space="PSUM"))
        ps_o = ctx.enter_context(tc.tile_pool(name="ps_o", bufs=1, space="PSUM"))

        def layernorm_tiles(src_tiles):
            """LN over 4 token tiles; returns 4 normalized tiles."""
            mv = stpool.tile([128, 8], F32, tag="mv", name="mv")
            for j in range(4):
                st6 = stpool.tile([128, 6], F32, tag="st6", name="st6")
                nc.vector.bn_stats(st6[:], src_tiles[j][:])
                nc.vector.bn_aggr(mv[:, 2 * j:2 * j + 2], st6[:])
            sg = stpool.tile([128, 4], F32, tag="sg", name="sg")
            rs = stpool.tile([128, 4], F32, tag="rs", name="rs")
            nmr = stpool.tile([128, 4], F32, tag="nmr", name="nmr")
            mv3 = mv[:].rearrange("p (j two) -> p j two", two=2)
            nc.scalar.activation(sg[:], mv3[:, :, 1], AF.Sqrt, bias=eps_sb[:, 0:1])
            nc.vector.reciprocal(rs[:], sg[:])
            for j in range(4):
                # nmr = -(mu * rsig)
                nc.vector.tensor_scalar(nmr[:, j:j + 1], mv3[:, j, 0].unsqueeze(-1),
                                        rs[:, j:j + 1], -1.0,
                                        op0=ALU.mult, op1=ALU.mult)
            xn_tiles = []
            for j in range(4):
                xn = xnpool.tile([128, D], F32, tag="xn", name="xn")
                nc.vector.tensor_scalar(xn[:], src_tiles[j][:], rs[:, j:j + 1],
                                        nmr[:, j:j + 1], op0=ALU.mult, op1=ALU.add)
                xn_tiles.append(xn)
            return xn_tiles

        def transpose_to_feature_major(xn_tiles):
            """4x [128, D] token-major -> 3x [128, 512] feature-major tiles."""
            xT = []
            for d in range(N_D):
                ps = ps_proj.tile([128, 512], F32, tag="ps", name="ps_t")
                for j in range(4):
                    nc.tensor.transpose(
                        ps[:, 128 * j:128 * (j + 1)],
                        xn_tiles[j][:, 128 * d:128 * (d + 1)],
                        identity[:],
                    )
                t = xTpool.tile([128, 512], MDT, tag=f"xT{d}", name=f"xT{d}")
                nc.any.tensor_copy(t[:], ps[:])
                xT.append(t)
            return xT

        # ======================= attention =======================
        for rep in range(n_reps):
          xsrc_d = x_d if rep == 0 else chain[rep - 1]
          xdst_d = out_d if rep == n_reps - 1 else chain[rep]
          x2_d = x2_ds[rep]
          for b in range(n_b):
            # ---- LN1 + transpose for this batch's 512 tokens ----
            xin = []
            for j in range(4):
                t0 = 128 * (4 * b + j)
                xt = xpool.tile([128, D], F32, tag="x", name="xt")
                nc.sync.dma_start(xt[:], xsrc_d[t0:t0 + 128, :])
                xin.append(xt)
            xn_tiles = layernorm_tiles(xin)
            xT = transpose_to_feature_major(xn_tiles)

            # ---- Q^T / K^T per head-pair: [54, 512] at partitions 0-53/64-117
            qt, kt = [], []
            for p in range(PAIRS):
                m = 118 if p < PAIRS - 1 else 54
                for (dst_list, w_sb, tg) in ((qt, wq_sb, "q"), (kt, wk_sb, "k")):
                    ps = ps_proj.tile([128, 512], F32, tag="ps", name="ps_qk")
                    for d in range(N_D):
                        nc.tensor.matmul(
                            ps[0:m, :],
                            cast(w_sb[d][:, 128 * p:128 * p + m]),
                            cast(xT[d][:]),
                            start=(d == 0), stop=(d == N_D - 1),
                        )
                    t = qkpool.tile([128, 512], MDT, tag=f"{tg}{p}", name=f"{tg}{p}")
                    nc.any.tensor_copy(t[0:m, :], ps[0:m, :])
                    dst_list.append(t)

            # ---- V token-major with ones column per head ----
            vt = []
            for j in range(4):
                ps = ps_proj.tile([128, 512], F32, tag="ps", name="ps_v")
                for d in range(N_D):
                    nc.tensor.matmul(
                        ps[:],
                        cast(xT[d][:, 128 * j:128 * (j + 1)]),
                        cast(wv_sb[d][:]),
                        start=(d == 0), stop=(d == N_D - 1),
                    )
                t = vpool.tile([128, 512], MDT, tag="v", name="vt")
                nc.any.memset(t[:].bitcast(F32), 1.0)
                src = ps[:, 0:448].rearrange("p (h c) -> p h c", h=H)[:, :, 1:55]
                dst = t[:, 0:448].rearrange("p (h c) -> p h c", h=H)[:, :, 1:55]
                nc.any.tensor_copy(dst, src)
                vt.append(t)

            # ---- per-head attention ----
            ot_b = [None] * H
            for h in range(H):
                p, sl = h // 2, 64 * (h % 2)
                sc = ps_sc.tile([128, 2048], F32, tag="sc", name="sc")
                for j in range(4):
                    # scores^T chunk j: [t=128, s in [128j, 512)]
                    nc.tensor.matmul(
                        sc[:, 512 * j + 128 * j: 512 * j + 512],
                        cast(kt[p][sl:sl + HS, 128 * j:128 * (j + 1)]),
                        cast(qt[p][sl:sl + HS, 128 * j:512]),
                        start=True, stop=True,
                    )
                eT = epool.tile([128, 2048], MDT, tag="eT", name="eT")
                for j in range(4):
                    lo, hi = 512 * j + 128 * j, 512 * j + 512
                    nc.scalar.activation(eT[:, lo:hi], sc[:, lo:hi], AF.Exp)
                # causal mask on the 4 diagonal blocks
                for j in range(4):
                    blk = eT[:, 640 * j: 640 * j + 128]
                    nc.vector.tensor_mul(blk, blk, trimask[:])
                # o^T accumulate over t-chunks; row 0 = softmax denominator
                ops = ps_o.tile([128, 512], F32, tag="o", name="ops")
                for j in range(4):
                    nc.tensor.matmul(
                        ops[0:55, 128 * j:512],
                        cast(vt[j][:, 64 * h: 64 * h + 55]),
                        cast(eT[:, 640 * j: 512 * j + 512]),
                        start=(j == 0), stop=(j == 3),
                    )
                r = rpool.tile([1, 512], F32, tag="r", name="r")
                rb = rpool.tile([55, 512], F32, tag="rb", name="rb")
                nc.vector.reciprocal_approx_fast(r[:], ops[0:1, :])
                nc.sync.dma_start(
                    rb[:], r[:].unsqueeze(1).to_broadcast([1, 55, 512]))
                ot = otpool.tile([55, 512], MDT, tag=f"ot{h}", name=f"ot{h}")
                nc.vector.tensor_mul(ot[:], ops[0:55, :], rb[:])
                ot_b[h] = ot

            # ---- attention out-proj + residual -> x2 (DRAM) ----
            for j in range(4):
                t0 = 128 * (4 * b + j)
                ps = ps_proj.tile([128, D], F32, tag="ps", name="ps_wo")
                for h in range(H):
                    nc.tensor.matmul(
                        ps[:],
                        cast(ot_b[h][:, 128 * j:128 * (j + 1)]),
                        cast(wo_sb[h][:]),
                        start=(h == 0), stop=(h == H - 1 and not has_bias_o),
                    )
                if has_bias_o:
                    nc.tensor.matmul(ps[:], cast(ones_sb[:]), cast(bo_sb[:]),
                                     start=False, stop=True)
                xr = xpool.tile([128, D], F32, tag="xr", name="xr")
                nc.sync.dma_start(xr[:], xsrc_d[t0:t0 + 128, :])
                x2t = opool.tile([128, D], F32, tag="out", name="x2t")
                nc.any.tensor_add(x2t[:], ps[:], xr[:])
                nc.sync.dma_start(x2_d[t0:t0 + 128, :], x2t[:])

          # ===================== FFN =====================
          for b in range(n_b):
            x2in = []
            for j in range(4):
                t0 = 128 * (4 * b + j)
                xt = xpool.tile([128, D], F32, tag="x", name="x2in")
                nc.sync.dma_start(xt[:], x2_d[t0:t0 + 128, :])
                x2in.append(xt)
            xn2_tiles = layernorm_tiles(x2in)
            xT2 = transpose_to_feature_major(xn2_tiles)

            # FFN2 accumulators: 4 tok-tiles share the 4 banks of one sc tile
            acc = ps_sc.tile([128, 2048], F32, tag="sc", name="acc")
            for g in range(2):
                hg = []
                for fi in range(9):
                    f = 9 * g + fi
                    ps = ps_proj.tile([128, 512], F32, tag="ps", name="ps_f1")
                    for d in range(N_D):
                        nc.tensor.matmul(
                            ps[:],
                            cast(w1_sb[d][:, 128 * f:128 * (f + 1)]),
                            cast(xT2[d][:]),
                            start=(d == 0), stop=(d == N_D - 1),
                        )
                    t = hpool.tile([128, 512], MDT, tag="hg", name="hg")
                    nc.scalar.activation(t[:], ps[:], AF.Gelu, bias=b1_sb[:, f:f + 1])
                    hg.append(t)
                for j in range(4):
                    for fi in range(9):
                        f = 9 * g + fi
                        nc.tensor.matmul(
                            acc[:, 512 * j: 512 * j + D],
                            cast(hg[fi][:, 128 * j:128 * (j + 1)]),
                            cast(w2_sb[f][:]),
                            start=(f == 0),
                            stop=(f == N_F - 1 and not has_bias_2),
                        )
            for j in range(4):
                t0 = 128 * (4 * b + j)
                if has_bias_2:
                    nc.tensor.matmul(acc[:, 512 * j:512 * j + D],
                                     cast(ones_sb[:]), cast(b2_sb[:]),
                                     start=False, stop=True)
                xr = xpool.tile([128, D], F32, tag="xr", name="xr2")
                nc.sync.dma_start(xr[:], x2_d[t0:t0 + 128, :])
                ot = opool.tile([128, D], F32, tag="out", name="outt")
                nc.any.tensor_add(ot[:], acc[:, 512 * j:512 * j + D], xr[:])
                nc.sync.dma_start(xdst_d[t0:t0 + 128, :], ot[:])

    nc.finalize()
    return nc


def preprocess(wq, bq, wk, bk, wv, bv, wo, bo, w1, b1, w2, b2,
               ln1_g, ln1_b, ln2_g, ln2_b):
    """Host-side folding: LN affine into weight matrices, attention scale into
    Q, V-bias into output bias; build padded/packed layouts."""
    f32 = np.float32
    args = [np.asarray(a, f32) for a in (wq, bq, wk, bk, wv, bv, wo, bo,
                                         w1, b1, w2, b2, ln1_g, ln1_b, ln2_g, ln2_b)]
    (wq, bq, wk, bk, wv, bv, wo, bo, w1, b1, w2, b2,
     ln1_g, ln1_b, ln2_g, ln2_b) = args
    scale = f32(HS) ** f32(-0.5)

    wq_pad = np.zeros((D, 512), f32)
    wk_pad = np.zeros((D, 512), f32)
    wv_pad = np.zeros((D, 512), f32)
    for h in range(H):
        wq_pad[:, 64 * h:64 * h + HS] = ln1_g[:, None] * wq[h] * scale
        wk_pad[:, 64 * h:64 * h + HS] = ln1_g[:, None] * wk[h]
        wv_pad[:, 64 * h + 1:64 * h + 1 + HS] = ln1_g[:, None] * wv[h]

    bq_eff = (bq + ln1_b @ wq).astype(f32)     # [H, HS]
    assert not np.any(bq_eff), "nonzero effective q bias not supported"
    # bk_eff shifts scores by a per-s constant -> cancelled by softmax; drop.

    bv_eff = (bv + ln1_b @ wv).astype(f32)     # [H, HS] -> folds into bo
    bo_eff = (bo + bv_eff.reshape(-1) @ wo).astype(f32)

    wo_pad = np.zeros((H, 55, D), f32)
    for h in range(H):
        wo_pad[h, 1:55, :] = wo[54 * h:54 * h + HS, :]

    w1_eff = (ln2_g[:, None] * w1).astype(f32)
    b1_eff = (b1 + ln2_b @ w1).astype(f32)
    b1c = np.ascontiguousarray(b1_eff.reshape(N_F, 128).T)   # [128, 18]

    return dict(
        wq_pad=to_fp32r(wq_pad), wk_pad=to_fp32r(wk_pad), wv_pad=to_fp32r(wv_pad),
        wo_pad=to_fp32r(wo_pad),
        w1=to_fp32r(w1_eff), b1c=b1c, w2=to_fp32r(w2),
        bo=to_fp32r(bo_eff.reshape(1, D)), b2=to_fp32r(b2.reshape(1, D)),
        has_bias_o=bool(np.any(bo_eff)), has_bias_2=bool(np.any(b2)),
    )


def kernel(**inputs):
    x = np.asarray(inputs["x"], np.float32)
    w = preprocess(
        inputs["wq"], inputs["bq"], inputs["wk"], inputs["bk"],
        inputs["wv"], inputs["bv"], inputs["wo"], inputs["bo"],
        inputs["w1"], inputs["b1"], inputs["w2"], inputs["b2"],
        inputs["ln1_g"], inputs["ln1_b"], inputs["ln2_g"], inputs["ln2_b"],
    )
    has_bo, has_b2 = w.pop("has_bias_o"), w.pop("has_bias_2")
    nc = build_program(n_b=B_LOC, has_bias_o=has_bo, has_bias_2=has_b2)

    core_ids = list(range(N_CORES))
    in_maps = []
    for c in core_ids:
        m = dict(w)
        m["x"] = np.ascontiguousarray(
            x[B_LOC * c:B_LOC * (c + 1)].reshape(B_LOC * S, D))
        in_maps.append(m)

    res = run_bass_kernel_spmd(nc, in_maps, core_ids)
    global LAST_RESULTS
    LAST_RESULTS = res
    out = np.concatenate(
        [res.results[i]["out"].reshape(B_LOC, S, D) for i in range(N_CORES)], axis=0
    )
    return out.astype(np.float32)

